# revision 70
# baseline (speedup 1.0000x reference)
"""Trainium2 Bass kernel for 2-layer GAT (nn_GAT_4861902979553).

Strategy (8 NeuronCores, SPMD):
  - Nodes sharded contiguously: core c owns rows [c*6250, (c+1)*6250).
  - Edges (incl. self-loops) partitioned by destination core, sorted by dst,
    grouped into 128-dst blocks; each block's edges are packed into 128-edge
    tiles that accumulate into a per-block PSUM via one-hot(alpha) matmuls.
  - Dense phase per layer computes an augmented row per node:
    [h_head0 | 1 | h_head1 | 1 | a_src... | pad] (fp16, 256B-multiple rows);
    slabs are AllGathered (Shared-output fast path) into a full gather table
    so the edge phase can fetch any source row locally. The a_dst values
    stay on-chip in a resident SBUF table (adres, fp16 per dst-block).
  - Per-edge rows fetched with dma_gather (int16 indices; edges of each block
    are split into two tile streams by source-node half so indices fit int16;
    the hi stream gathers from a table offset of 32768 rows).
  - Per-edge a_dst is computed on-chip (no gather): a transposed one-hot
    ohT[d, e] is built per gather group from static dstrow data (K=1
    broadcast matmul + is_equal against a per-partition iota), then
    adst_e = ohT.T @ adres[block] via small matmuls.
  - Attention: alpha = exp(lrelu(asrc+adst) - 8); the -8 shift keeps exp in
    fp16 range and cancels in softmax normalization.
  - Aggregation: one PSUM tile per block holds all heads; region
    [129h:129h+129] += onehot_alpha.T @ [h_head | 1]; col 129h+128
    accumulates the softmax denominator. PE matmul start=True resets the
    whole PSUM tile, so only the first matmul of a block carries start.
  - Layer-1 epilogue divides, applies ReLU, and PE-transposes o1 directly
    into the layer-2 dense lhsT tiles (no DRAM roundtrip), letting dense2
    pipeline per-block behind the layer-1 edge phase.
"""

import numpy as np

# Problem constants (hardcoded per harness contract)
N_NODES = 50000
N_EDGES = 800000
IN_FEATS = 256
HIDDEN = 128
NEG_SLOPE = 0.2
N_CORES = 8
P = 128
HALF = 32768  # int16 index limit; src-node split point
SHIFT = 8.0  # exp shift; cancels in softmax, keeps fp16 in range
import os as _os_mod
G_TILES = int(_os_mod.environ.get("K_GT", "8"))  # edge tiles per gather group
# (default 1024 idxs/call, the SWDGE ring limit at 16KB scratch; K_GT>8 needs
# a matching dynamic_dma_scratch_size bump below)
ROW1 = 384  # layer-1 gather row (260 used, padded to 768B)
L2SLIM = int(_os_mod.environ.get("K_L2SLIM", "0"))
# L2SLIM=1: 256B layer-2 gather rows (h only; asrc2 via on-chip DVE dot,
# denominator via ones-column matmul). Measured slower than the 512B row
# on HW despite halved bytes, so default off.
ROW2 = 128 if L2SLIM else 256

F16 = np.float16


# --------------------------------------------------------------------------
# Host-side planning
# --------------------------------------------------------------------------

def _wrap_idx(flat):
    """dma_gather index layout: idxs[p, s] = flat[s*16 + p], replicated x8."""
    wrap = flat.reshape(-1, 16).T
    return np.tile(wrap, (8, 1)).astype(np.int16)


def _plan_edges(edge_index, n_nodes, n_cores=N_CORES, g_tiles=G_TILES):
    nsh = n_nodes // n_cores
    nblk = (nsh + P - 1) // P
    src = np.asarray(edge_index[0], np.int64)
    dst = np.asarray(edge_index[1], np.int64)
    loop = np.arange(n_nodes, dtype=np.int64)
    src = np.concatenate([src, loop])
    dst = np.concatenate([dst, loop])
    core = dst // nsh

    # per (core, block, half) sorted edge lists
    counts = np.zeros((n_cores, nblk, 2), np.int64)
    ecore = []
    for c in range(n_cores):
        m = core == c
        s_c = src[m]
        d_c = dst[m] - c * nsh
        hf = (s_c >= HALF).astype(np.int64)
        key = (d_c // P) * 2 + hf  # sort by (block, half), then dst
        o = np.lexsort((d_c, key))
        s_c, d_c, hf = s_c[o], d_c[o], hf[o]
        bh = np.bincount((d_c // P) * 2 + hf, minlength=nblk * 2)
        counts[c] = bh.reshape(nblk, 2)
        ecore.append((s_c, d_c))

    tiles_bh = np.maximum(0, -(-counts // P)).max(axis=0)  # [nblk, 2]
    tiles_bh[:, 0] = np.maximum(tiles_bh[:, 0], tiles_bh.sum(1) == 0)
    tiles_pb = tiles_bh.sum(1)
    T = int(tiles_pb.sum())
    blk_start = np.concatenate([[0], np.cumsum(tiles_pb)])[:-1].astype(int)

    # static per-tile structure (identical on every core)
    half = np.zeros(T, np.int64)
    for b in range(nblk):
        half[blk_start[b] + tiles_bh[b, 0]:blk_start[b] + tiles_pb[b]] = 1
    stream_tiles = [np.nonzero(half == s)[0] for s in range(2)]
    t_sizes = [len(st) for st in stream_tiles]
    stream_pos = np.zeros(T, np.int64)
    for s in range(2):
        stream_pos[stream_tiles[s]] = np.arange(t_sizes[s])

    gsrc = np.zeros((n_cores, P, T), np.int64)
    dstcol = np.full((n_cores, P, T), -1.0, np.float32)
    for c in range(n_cores):
        s_c, d_c = ecore[c]
        sob = np.concatenate([[0], np.cumsum(counts[c].ravel())]).astype(int)
        for b in range(nblk):
            for hf in range(2):
                e0, e1 = sob[b * 2 + hf], sob[b * 2 + hf + 1]
                cnt = e1 - e0
                if cnt == 0:
                    continue
                t0 = blk_start[b] + (tiles_bh[b, 0] if hf else 0)
                o = np.arange(cnt)
                tt, pp = t0 + o // P, o % P
                gsrc[c, pp, tt] = s_c[e0:e1] - hf * HALF
                dstcol[c, pp, tt] = (d_c[e0:e1] - b * P).astype(np.float32)

    # per-stream wrapped int16 index arrays, grouped per g_tiles
    gsrc_w = []
    for s in range(2):
        st = stream_tiles[s]
        gcols = []
        for g0 in range(0, len(st), g_tiles):
            tsel = st[g0:g0 + g_tiles]
            gs = gsrc[:, :, tsel]  # [c, P, gw]
            gw = len(tsel)
            # flat index i = t_rel*128 + p
            gflat = gs.transpose(0, 2, 1).reshape(n_cores, gw * P)
            gcols.append(np.stack([_wrap_idx(gflat[c]) for c in range(n_cores)]))
        if gcols:
            gsrc_w.append(np.concatenate(gcols, axis=2))
        else:
            gsrc_w.append(np.zeros((n_cores, P, 0), np.int16))

    # dstrow: per-tile dst-local values as a flat row, for the transposed
    # one-hot build (bcast-matmul + is_equal against a per-partition iota)
    dstrow = dstcol.transpose(0, 2, 1).reshape(n_cores, 1, T * P).astype(F16)

    return dict(
        nsh=nsh, nblk=nblk, T=T, t_sizes=t_sizes,
        tiles_pb=tiles_pb.astype(int), blk_start=blk_start,
        half=half, stream_pos=stream_pos, stream_tiles=stream_tiles,
        gsrc_w=gsrc_w, dstcol=dstcol, dstrow=dstrow,
    )


def _prep_weights(W1, att_src1, att_dst1, W2, att_src2, att_dst2):
    W1t = np.asarray(W1, np.float32).T  # [256, 256]
    W1aug = np.zeros((IN_FEATS, 262), np.float32)
    W1aug[:, 0:128] = W1t[:, 0:128]
    W1aug[:, 129:257] = W1t[:, 128:256]
    a_s, a_d = np.asarray(att_src1, np.float32), np.asarray(att_dst1, np.float32)
    for k in range(2):
        W1aug[:, 258 + k] = W1t[:, k * 128:(k + 1) * 128] @ a_s[0, k]
        W1aug[:, 260 + k] = W1t[:, k * 128:(k + 1) * 128] @ a_d[0, k]
    W2t = np.asarray(W2, np.float32).T  # [256, 128]
    W2aug = np.zeros((IN_FEATS, 131), np.float32)
    W2aug[:, 0:128] = W2t
    W2aug[:, 129] = W2t @ np.asarray(att_src2, np.float32)[0, 0]
    W2aug[:, 130] = W2t @ np.asarray(att_dst2, np.float32)[0, 0]
    return W1aug.astype(F16), W2aug.astype(F16)


# --------------------------------------------------------------------------
# Device program
# --------------------------------------------------------------------------

def _build_program(n_nodes, plan, phases=6, sim=False):
    """phases: 1=dense1, 2=+ag1, 3=+edge1, 4=+transpose+dense2, 5=+ag2, 6=full

    sim=True replaces the AllGathers with local DMA copies so the module has
    no collectives and can run under TimelineSim (single-core cost model)."""
    import concourse.bass as bass
    import concourse.bacc as bacc
    import concourse.mybir as mybir
    import concourse.tile as tile

    dt = mybir.dt
    nsh, nblk, T = plan["nsh"], plan["nblk"], plan["T"]
    tiles_pb, blk_start = plan["tiles_pb"], plan["blk_start"]
    half, stream_pos, t_sizes = plan["half"], plan["stream_pos"], plan["t_sizes"]
    npad = nblk * P

    import os as _os0
    NQ = int(_os0.environ.get("K_NQ", "1"))
    nc = bacc.Bacc("TRN2", target_bir_lowering=False, debug=False,
                   enable_asserts=True, num_devices=N_CORES,
                   num_swdge_queues=NQ,
                   dynamic_dma_scratch_size=G_TILES * P * 16)

    # ---- I/O ----
    xT = nc.dram_tensor("xT", [IN_FEATS, npad], dt.float16, kind="ExternalInput")
    w1 = nc.dram_tensor("W1aug", [IN_FEATS, 262], dt.float16, kind="ExternalInput")
    w2 = nc.dram_tensor("W2aug", [IN_FEATS, 131], dt.float16, kind="ExternalInput")
    gsrc_d = [nc.dram_tensor(f"gsrc{s}", [P, max(1, 8 * t_sizes[s])], dt.int16,
                             kind="ExternalInput") for s in range(2)]
    dstrow_d = [nc.dram_tensor(f"dstrow{s}", [1, max(P, t_sizes[s] * P)],
                               dt.float16, kind="ExternalInput")
                for s in range(2)]
    dstcol_d = nc.dram_tensor("dstcol", [P, T], dt.float32, kind="ExternalInput")
    asrc2_d = nc.dram_tensor("asrc2", [1, HIDDEN], dt.float16, kind="ExternalInput")
    out_d = nc.dram_tensor("out", [nsh, HIDDEN], dt.float32, kind="ExternalOutput")
    import os
    dbg = int(os.environ.get("K_DEBUG", "0"))
    if dbg:
        dbg_h1 = nc.dram_tensor("dbg_h1", [nsh, ROW1], dt.float16, kind="ExternalOutput")
        dbg_o1 = nc.dram_tensor("dbg_o1", [nsh, 256], dt.float16, kind="ExternalOutput")
        dbg_h2 = nc.dram_tensor("dbg_h2", [nsh, ROW2], dt.float16, kind="ExternalOutput")
        dbg_o1T = nc.dram_tensor("dbg_o1T", [IN_FEATS, npad], dt.float16, kind="ExternalOutput")

    # ---- internal DRAM ----
    h1_slab = nc.dram_tensor("h1_slab", [nsh, ROW1], dt.float16)
    tab1 = nc.dram_tensor("tab1", [n_nodes, ROW1], dt.float16,
                          addr_space="Shared")
    h2_slab = nc.dram_tensor("h2_slab", [nsh, ROW2], dt.float16)
    tab2 = nc.dram_tensor("tab2", [n_nodes, ROW2], dt.float16,
                          addr_space="Shared")

    groups = [list(range(N_CORES))]

    with tile.TileContext(nc) as tc:
        import contextlib
        ctx = contextlib.ExitStack()
        with ctx:
            res = ctx.enter_context(tc.tile_pool(name="res", bufs=1))
            dense_ps = ctx.enter_context(tc.tile_pool(name="dps", bufs=1, space="PSUM"))
            tps = ctx.enter_context(tc.tile_pool(name="tps", bufs=1, space="PSUM"))
            dense_sb = ctx.enter_context(tc.tile_pool(name="dsb", bufs=2))
            gath = ctx.enter_context(
                tc.tile_pool(name="gath", bufs=3 if G_TILES <= 8 else 2))
            alph = ctx.enter_context(tc.tile_pool(name="alph", bufs=3))
            sal = ctx.enter_context(tc.tile_pool(name="sal", bufs=4))
            ohp = ctx.enter_context(tc.tile_pool(name="ohp", bufs=2))
            blk_ps = ctx.enter_context(tc.tile_pool(name="bps", bufs=2, space="PSUM"))
            obp = ctx.enter_context(tc.tile_pool(name="obp", bufs=2, space="PSUM"))
            abp = ctx.enter_context(tc.tile_pool(name="abp", bufs=2, space="PSUM"))
            epi = ctx.enter_context(tc.tile_pool(name="epi", bufs=2))

            # ---- resident tiles ----
            xT_sb = [res.tile([P, npad], dt.float16, tag=f"xT{k}", name=f"xT{k}")
                     for k in range(2)]
            w1_sb = [res.tile([P, 262], dt.float16, tag=f"w1_{k}", name=f"w1_{k}")
                     for k in range(2)]
            w2_sb = [res.tile([P, 131], dt.float16, tag=f"w2_{k}", name=f"w2_{k}")
                     for k in range(2)]
            gsrc_sb = [res.tile([P, max(1, 8 * t_sizes[s])], dt.int16,
                                tag=f"gsrc{s}", name=f"gsrc{s}") for s in range(2)]
            dstcol_sb = res.tile([P, T], dt.float32, tag="dstcol", name="dstcol")
            iota_i = res.tile([P, P], dt.int16, tag="iota_i", name="iota_i")
            iota_f = res.tile([P, P], dt.float16, tag="iota_f", name="iota_f")
            iota_ci = res.tile([P, 1], dt.int16, tag="iota_ci", name="iota_ci")
            iota_c = res.tile([P, 1], dt.float32, tag="iota_c", name="iota_c")
            ones1 = res.tile([1, P], dt.float16, tag="ones1", name="ones1")
            adres1 = res.tile([P, nblk * 2], dt.float16, tag="adres1",
                              name="adres1")
            adres2 = res.tile([P, nblk], dt.float16, tag="adres2", name="adres2")
            o1T_sb = [res.tile([P, npad], dt.float16, tag=f"o1T{k}", name=f"o1T{k}")
                      for k in range(2)]
            ident = res.tile([P, P], dt.float16, tag="ident", name="ident")
            nshift = res.tile([P, 1], dt.float32, tag="nshift", name="nshift")
            ones_col = res.tile([P, 1], dt.float16, tag="ones_col", name="ones_col")
            asrc2row = res.tile([1, HIDDEN], dt.float16, tag="asrc2row",
                                name="asrc2row")
            att2b = res.tile([P, HIDDEN], dt.float16, tag="att2b", name="att2b")

            for k in range(2):
                nc.sync.dma_start(out=xT_sb[k][:], in_=xT[k * P:(k + 1) * P, :])
                nc.sync.dma_start(out=w1_sb[k][:], in_=w1[k * P:(k + 1) * P, :])
                nc.sync.dma_start(out=w2_sb[k][:], in_=w2[k * P:(k + 1) * P, :])
            for s in range(2):
                nc.sync.dma_start(out=gsrc_sb[s][:], in_=gsrc_d[s][:, :])
            nc.sync.dma_start(out=dstcol_sb[:], in_=dstcol_d[:, :])
            nc.gpsimd.iota(iota_i[:], pattern=[[1, P]], channel_multiplier=0)
            nc.vector.tensor_copy(out=iota_f[:], in_=iota_i[:])
            nc.gpsimd.iota(iota_ci[:], pattern=[[0, 1]], channel_multiplier=1)
            nc.vector.tensor_copy(out=iota_c[:], in_=iota_ci[:])
            nc.vector.memset(ones1[:], 1.0)
            nc.vector.tensor_scalar(
                out=ident[:], in0=iota_f[:], scalar1=iota_c[:, 0:1],
                scalar2=None, op0=mybir.AluOpType.is_equal)
            nc.vector.memset(nshift[:], -SHIFT)
            nc.vector.memset(ones_col[:], 1.0)
            # keep debug modes (K_EDGE_SUB<4 / K_NO_*) valid: these are
            # normally fully overwritten before use
            for k in range(2):
                nc.vector.memset(o1T_sb[k][:], 0.0)
            nc.vector.memset(adres1[:], 0.0)
            nc.vector.memset(adres2[:], 0.0)
            nc.sync.dma_start(out=asrc2row[:], in_=asrc2_d[:, :])
            pt0 = tps.tile([P, P], dt.float32, tag="tps", name="tps")
            nc.tensor.matmul(out=pt0[:], lhsT=ones1[:, :], rhs=asrc2row[:, :],
                             start=True, stop=True)
            nc.scalar.activation(out=att2b[:], in_=pt0[:],
                                 func=mybir.ActivationFunctionType.Copy)

            def dense_layer(w_sb, ncols, rowlen, stglen, slab, adres, lhsT,
                            acols, ones_at):
                writes = []
                nads = acols[1] - acols[0]  # a_dst column count
                for nb in range(nblk):
                    rows = min(P, nsh - nb * P)
                    ps = dense_ps.tile([P, ncols], dt.float32, tag="dps", name="dps")
                    for kc in range(2):
                        nc.tensor.matmul(
                            ps[:], lhsT=lhsT[kc][:, nb * P:(nb + 1) * P],
                            rhs=w_sb[kc][:], start=(kc == 0), stop=(kc == 1))
                    stg = dense_sb.tile([P, stglen], dt.float16, tag="dstg", name="dstg")
                    nc.vector.tensor_copy(out=stg[:, 0:rowlen], in_=ps[:, 0:rowlen])
                    for oc in ones_at:
                        nc.vector.memset(stg[:, oc:oc + 1], 1.0)
                    if stglen > rowlen:
                        nc.vector.memset(stg[:, rowlen:stglen], 0.0)
                    nc.vector.tensor_copy(
                        out=adres[:, nb * nads:(nb + 1) * nads],
                        in_=ps[:, acols[0]:acols[1]])
                    writes.append(nc.sync.dma_start(
                        out=slab[nb * P:nb * P + rows, :], in_=stg[:rows, :]))
                    if dbg and stglen == ROW2 and ncols == 131:
                        nc.sync.dma_start(out=dbg_h2[nb * P:nb * P + rows, :],
                                          in_=stg[:rows, :])
                    if dbg and stglen == ROW1 and ncols == 262:
                        nc.sync.dma_start(out=dbg_h1[nb * P:nb * P + rows, :],
                                          in_=stg[:rows, :])
                return writes

            def edge_layer(tab, heads, rowlen, asrc_off, epilogue, barrier):
                from bass_rust import add_dep_helper
                import os
                sub = int(os.environ.get("K_EDGE_SUB", "4"))
                no_abuf = int(os.environ.get("K_NO_ABUF", "0"))
                no_gbuf = int(os.environ.get("K_NO_GBUF", "0"))
                if no_abuf or no_gbuf:
                    sub = 0
                adres = adres1 if heads == 2 else adres2
                stream_order = [np.nonzero(half == ss)[0] for ss in range(2)]
                sbuf = [None, None]  # per-stream current (gbuf, ale)
                psum = {}
                qctr = [0]  # round-robin gather queue assignment
                for t in range(T):
                    s, sp = int(half[t]), int(stream_pos[t])
                    g, j = divmod(sp, G_TILES)
                    if j == 0:
                        gw = min(G_TILES, t_sizes[s] - g * G_TILES)
                        tgrp = stream_order[s][g * G_TILES:g * G_TILES + gw]
                        gbuf = gath.tile([P, gw, rowlen], dt.float16,
                                         tag=f"gbuf{s}", name=f"gbuf{s}")
                        if not no_gbuf:
                            tbase = tab[s * HALF:min(n_nodes, (s + 1) * HALF), :]
                            gi = nc.gpsimd.dma_gather(
                                out_ap=gbuf[:], in_ap=tbase,
                                idxs_ap=gsrc_sb[s][:, g * G_TILES * 8:(g * G_TILES + gw) * 8],
                                num_idxs=gw * P, num_idxs_reg=gw * P, elem_size=rowlen,
                                queue_num=qctr[0] % NQ)
                            qctr[0] += 1
                            add_dep_helper(gi.ins, barrier.ins, sync=True,
                                           reason="gather after table ready")
                        if no_abuf or no_gbuf:
                            sbuf[s] = (gbuf, None)
                            continue
                        # transposed one-hot ohT[d, e] for the whole group,
                        # from static dstrow via K=1 bcast matmul + is_equal
                        drow = gath.tile([1, gw * P], dt.float16,
                                         tag=f"drow{s}", name=f"drow{s}")
                        nc.sync.dma_start(
                            out=drow[:],
                            in_=dstrow_d[s][0:1, g * G_TILES * P:
                                            g * G_TILES * P + gw * P])
                        ohT = ohp.tile([P, gw * P], dt.float16,
                                       tag=f"ohT{s}", name=f"ohT{s}")
                        for cb in range(0, gw * P, 512):
                            cw = min(512, gw * P - cb)
                            pb = obp.tile([P, 512], dt.float32,
                                          tag="obp_pb", name=f"pb{s}")
                            nc.tensor.matmul(
                                out=pb[:, 0:cw], lhsT=ones1[:, :],
                                rhs=drow[0:1, cb:cb + cw],
                                start=True, stop=True)
                            nc.vector.tensor_scalar(
                                out=ohT[:, cb:cb + cw],
                                in0=pb[:, 0:cw], scalar1=iota_c[:, 0:1],
                                scalar2=None, op0=mybir.AluOpType.is_equal)
                        # adst_e[e, h] = sum_d ohT[d, e] * adres[d, b*heads+h]
                        pa = abp.tile([P, G_TILES * heads], dt.float32,
                                      tag="abp_pa", name=f"pa{s}")
                        for jj in range(gw):
                            bb = int(np.searchsorted(
                                blk_start, int(tgrp[jj]), side="right")) - 1
                            nc.tensor.matmul(
                                out=pa[:, jj * heads:(jj + 1) * heads],
                                lhsT=ohT[:, jj * P:(jj + 1) * P],
                                rhs=adres[:, bb * heads:(bb + 1) * heads],
                                start=True, stop=True)
                        # alpha = exp(lrelu(asrc + adst) - SHIFT); lrelu+exp on
                        # the (mostly idle) ACT engine to keep DVE short
                        tsum = alph.tile([P, gw, heads], dt.float32,
                                         tag=f"tsum{s}", name=f"tsum{s}")
                        if asrc_off is None:
                            # on-chip asrc: rowwise dot of gathered h with att
                            asr = alph.tile([P, gw, heads], dt.float32,
                                            tag=f"asr{s}", name=f"asr{s}")
                            tm = alph.tile([P, gw, P], dt.float16,
                                           tag=f"tm{s}", name=f"tm{s}")
                            for jj in range(gw):
                                nc.vector.tensor_tensor(
                                    out=tm[:, jj, :], in0=gbuf[:, jj, 0:P],
                                    in1=att2b[:, :], op=mybir.AluOpType.mult)
                            nc.vector.tensor_reduce(
                                out=asr[:], in_=tm[:],
                                axis=mybir.AxisListType.X,
                                op=mybir.AluOpType.add)
                            nc.vector.tensor_tensor(
                                out=tsum[:], in0=asr[:], in1=pa[:, 0:gw * heads],
                                op=mybir.AluOpType.add)
                        else:
                            nc.vector.tensor_tensor(
                                out=tsum[:],
                                in0=gbuf[:, :, asrc_off:asrc_off + heads],
                                in1=pa[:, 0:gw * heads],
                                op=mybir.AluOpType.add)
                        lr = alph.tile([P, gw, heads], dt.float32,
                                       tag=f"lr{s}", name=f"lr{s}")
                        if int(os.environ.get("K_ACT_LRELU", "0")):
                            nc.scalar.activation(
                                out=lr[:], in_=tsum[:],
                                func=mybir.ActivationFunctionType.Lrelu,
                                alpha=NEG_SLOPE)
                        else:
                            tng = alph.tile([P, gw, heads], dt.float32,
                                            tag=f"tng{s}", name=f"tng{s}")
                            nc.vector.tensor_scalar(
                                out=tng[:], in0=tsum[:], scalar1=NEG_SLOPE,
                                scalar2=None, op0=mybir.AluOpType.mult)
                            nc.vector.tensor_tensor(
                                out=lr[:], in0=tsum[:], in1=tng[:],
                                op=mybir.AluOpType.max)
                        ale = alph.tile([P, gw, heads], dt.float32,
                                        tag=f"ale{s}", name=f"ale{s}")
                        nc.scalar.activation(
                            out=ale[:], in_=lr[:],
                            func=mybir.ActivationFunctionType.Exp, bias=nshift[:])
                        sbuf[s] = (gbuf, ale)
                    gbuf, ale = sbuf[s]
                    if sub < 1:
                        continue
                    b = int(np.searchsorted(blk_start, t, side="right")) - 1
                    first = t == blk_start[b]
                    last = t == blk_start[b] + tiles_pb[b] - 1
                    if first and sub >= 3:
                        # one psum tile for all heads+denominators; PE start
                        # resets the whole tile, so only the first matmul of
                        # the block carries start=True
                        psum = blk_ps.tile([P, heads * 129], dt.float32,
                                           tag="pb", name="pb")
                    for h in range(heads):
                        if sub < 2:
                            continue
                        sa = sal.tile([P, P], dt.float16, tag=f"sa{h}", name=f"sa{h}")
                        nc.vector.tensor_scalar(
                            out=sa[:], in0=iota_f[:],
                            scalar1=dstcol_sb[:, t:t + 1],
                            scalar2=ale[:, j, h:h + 1],
                            op0=mybir.AluOpType.is_equal, op1=mybir.AluOpType.mult)
                        if sub < 3:
                            continue
                        if asrc_off is None:
                            # h-only row: aggregate h, denominator via ones col
                            nc.tensor.matmul(
                                out=psum[:, 129 * h + P:129 * h + P + 1],
                                lhsT=sa[:], rhs=ones_col[:, :],
                                start=first and h == 0, stop=False)
                            nc.tensor.matmul(
                                out=psum[:, 129 * h:129 * h + P], lhsT=sa[:],
                                rhs=gbuf[:, j, 0:P],
                                start=False, stop=last and h == heads - 1)
                        else:
                            nc.tensor.matmul(
                                out=psum[:, 129 * h:129 * h + 129], lhsT=sa[:],
                                rhs=gbuf[:, j, 129 * h:129 * h + 129],
                                start=first and h == 0,
                                stop=last and h == heads - 1)
                    if last and sub >= 4:
                        epilogue(b, psum)

            def bail():
                dummy = epi.tile([P, HIDDEN], dt.float32, tag="dummy", name="dummy")
                nc.vector.memset(dummy[:], 0.0)
                for nb in range(nblk):
                    rows = min(P, nsh - nb * P)
                    nc.scalar.dma_start(out=out_d[nb * P:nb * P + rows, :],
                                        in_=dummy[:rows, :])

            # ---------------- Layer 1 ----------------
            from bass_rust import add_dep_helper as _adh
            import os as _os
            amp_cc = int(_os.environ.get("K_AMP_CC", "1"))
            amp_edge = int(_os.environ.get("K_AMP_EDGE", "1"))
            d1w = dense_layer(w1_sb, 262, 260, ROW1, h1_slab, adres1, xT_sb,
                              acols=(260, 262), ones_at=(128, 257))
            cc1 = None
            if phases >= 2:
                for _rep in range(amp_cc):
                    prev = cc1
                    if sim:
                        cc1 = nc.sync.dma_start(out=tab1[0:nsh, :], in_=h1_slab[:, :])
                    else:
                        cc1 = nc.gpsimd.collective_compute(
                            "AllGather", mybir.AluOpType.bypass, replica_groups=groups,
                            ins=[h1_slab.ap()], outs=[tab1.ap()])
                    for w in d1w:
                        _adh(cc1.ins, w.ins, sync=True, reason="allgather after dense writes")
                    if prev is not None:
                        _adh(cc1.ins, prev.ins, sync=True, reason="amp chain")

            def epi1(b, psum):
                rows = min(P, nsh - b * P)
                rc = epi.tile([P, 2], dt.float32, tag="rc", name="rc")
                dn = epi.tile([P, 2], dt.float32, tag="dn", name="dn")
                for h in range(2):
                    nc.vector.tensor_scalar(
                        out=dn[:, h:h + 1], in0=psum[:, 129 * h + 128:129 * h + 129],
                        scalar1=1e-6,
                        scalar2=None, op0=mybir.AluOpType.max)
                nc.vector.reciprocal(out=rc[:], in_=dn[:])
                o1s = epi.tile([P, 256], dt.float16, tag="o1s", name="o1s")
                for h in range(2):
                    nc.vector.tensor_scalar(
                        out=o1s[:, h * 128:(h + 1) * 128],
                        in0=psum[:, 129 * h:129 * h + 128],
                        scalar1=rc[:, h:h + 1], scalar2=0.0,
                        op0=mybir.AluOpType.mult, op1=mybir.AluOpType.max)
                if phases >= 4:
                    # PE-transpose o1s into the layer-2 dense lhsT tiles
                    for k in range(2):
                        pt = tps.tile([P, P], dt.float32, tag="tps", name="tps")
                        nc.tensor.matmul(
                            out=pt[:], lhsT=o1s[:, k * P:(k + 1) * P],
                            rhs=ident[:], start=True, stop=True)
                        nc.scalar.activation(
                            out=o1T_sb[k][:, b * P:(b + 1) * P], in_=pt[:],
                            func=mybir.ActivationFunctionType.Copy)
                if dbg:
                    nc.scalar.dma_start(out=dbg_o1[b * P:b * P + rows, :],
                                        in_=o1s[:rows, :])

            if phases < 3:
                bail()
            if phases >= 3:
                for _rep in range(amp_edge):
                    edge_layer(tab1, 2, ROW1, 258, epi1, cc1)

            cc2 = None
            if phases >= 4:
                if dbg:
                    for k in range(2):
                        nc.sync.dma_start(out=dbg_o1T[k * P:(k + 1) * P, :],
                                          in_=o1T_sb[k][:])
                # ---------------- Layer 2 ----------------
                d2w = dense_layer(w2_sb, 131, 128 if L2SLIM else 130, ROW2,
                                  h2_slab, adres2, o1T_sb,
                                  acols=(130, 131),
                                  ones_at=() if L2SLIM else (128,))
            if phases >= 5:
                for _rep in range(amp_cc):
                    prev = cc2
                    if sim:
                        cc2 = nc.sync.dma_start(out=tab2[0:nsh, :], in_=h2_slab[:, :])
                    else:
                        cc2 = nc.gpsimd.collective_compute(
                            "AllGather", mybir.AluOpType.bypass, replica_groups=groups,
                            ins=[h2_slab.ap()], outs=[tab2.ap()])
                    for w in d2w:
                        _adh(cc2.ins, w.ins, sync=True, reason="allgather2 after dense writes")
                    if prev is not None:
                        _adh(cc2.ins, prev.ins, sync=True, reason="amp chain")

            def epi2(b, psum):
                rows = min(P, nsh - b * P)
                rc = epi.tile([P, 1], dt.float32, tag="rc2", name="rc2")
                dn = epi.tile([P, 1], dt.float32, tag="dn2", name="dn2")
                nc.vector.tensor_scalar(
                    out=dn[:], in0=psum[:, 128:129], scalar1=1e-6,
                    scalar2=None, op0=mybir.AluOpType.max)
                nc.vector.reciprocal(out=rc[:], in_=dn[:])
                os_ = epi.tile([P, 128], dt.float32, tag="os", name="os")
                nc.vector.tensor_scalar(
                    out=os_[:], in0=psum[:, 0:128], scalar1=rc[:, 0:1],
                    scalar2=None, op0=mybir.AluOpType.mult)
                nc.scalar.dma_start(out=out_d[b * P:b * P + rows, :],
                                    in_=os_[:rows, :])

            if phases >= 6:
                for _rep in range(amp_edge):
                    edge_layer(tab2, 1, ROW2, None if L2SLIM else 129,
                               epi2, cc2)
            elif phases >= 3:
                bail()

    nc.compile()
    return nc


# --------------------------------------------------------------------------
# Host entry
# --------------------------------------------------------------------------

def _make_in_maps(inputs, plan):
    x = np.asarray(inputs["x"], np.float32)
    W1aug, W2aug = _prep_weights(
        inputs["W1"], inputs["att_src1"], inputs["att_dst1"],
        inputs["W2"], inputs["att_src2"], inputs["att_dst2"])
    nsh, nblk = plan["nsh"], plan["nblk"]
    npad = nblk * P
    in_maps = []
    for c in range(N_CORES):
        xs = x[c * nsh:(c + 1) * nsh]
        xT = np.zeros((IN_FEATS, npad), F16)
        xT[:, :nsh] = xs.T.astype(F16)
        m = {"xT": xT, "W1aug": W1aug, "W2aug": W2aug,
             "dstcol": plan["dstcol"][c],
             "asrc2": np.asarray(inputs["att_src2"], np.float32
                                 ).reshape(1, HIDDEN).astype(F16)}
        ts = plan["t_sizes"]
        st = plan["stream_tiles"]
        dstcol_c = plan["dstcol"][c]  # [P, T]
        for s in range(2):
            gw = plan["gsrc_w"][s][c]
            if gw.shape[1] == 0:
                gw = np.zeros((P, 1), np.int16)
            m[f"gsrc{s}"] = gw
            # dstrow in stream order: [1, t_sizes[s]*P]
            dr = dstcol_c[:, st[s]].T.reshape(1, -1).astype(F16)
            if dr.shape[1] < P:
                dr = np.full((1, P), -1.0, F16)
            m[f"dstrow{s}"] = dr
        in_maps.append(m)
    return in_maps


def run(inputs, trace=False, **spmd_kwargs):
    assert float(np.abs(np.asarray(inputs["b1"])).max()) == 0.0, "b1 must be 0"
    plan = _plan_edges(inputs["edge_index"], N_NODES)
    nc = _build_program(N_NODES, plan)
    in_maps = _make_in_maps(inputs, plan)
    from concourse import bass_utils
    res = bass_utils.run_bass_kernel_spmd(
        nc, in_maps, core_ids=list(range(N_CORES)), trace=trace, **spmd_kwargs)
    out = np.concatenate([res.results[c]["out"] for c in range(N_CORES)], axis=0)
    out = (out + np.asarray(inputs["b2"], np.float32)[None, :]).astype(np.float32)
    return out, res


def kernel(**inputs):
    return run(inputs)[0]

